# revision 23
# baseline (speedup 1.0000x reference)
"""FFF (fast feedforward / MoE tree-routing) Trainium2 kernel, v2.

Strategy (8 NeuronCores, SPMD, two launches):
  Launch 1 - routing, data-parallel over batch: each core routes 1024 samples
    through the depth-11 plane tree. Levels 0..7 are evaluated densely with
    fp32 matmuls (255 nodes); per-sample select via i16 iota/is_equal masks
    and bf16 one-hot reduce on VectorE; PSUM->SBUF score copies (+node bias)
    run on GpSimd. Levels 8..10 gather each sample's [w|b] node row straight
    from DRAM with per-c-tile indirect DMAs (SBUF-resident int32 offsets, no
    DRAM index round-trip) and reduce with fused scalar_tensor_tensor dots
    on VectorE; 8 c-tile chains overlap gather DMA with other chains' dots.
  Host - slot assignment: samples grouped by leaf expert; leaves sharded
    expert-parallel 256/core; groups sorted by occupancy per core so one
    shared capacity plan (per-rank max across cores) packs all cores into
    one NEFF with ~1.1x slot overhead.
  Launch 2 - expert MLP, expert-parallel, bf16: per 8-expert group one
    [768x128] @ [768xcap] bf16 matmul chain computes all 8 experts' h lanes,
    relu+bias on ScalarE, lane-mask on VectorE, then h.T @ W2 in bf16.
    Outputs pack into [<=128, 768] tiles (copies alternate Vector/Scalar)
    and store bf16. Weights stream through SBUF once per core (12.6 MB).
  Host - scatter output rows back to sample order.
"""

import contextlib
import numpy as np
import ml_dtypes

import concourse.bacc as bacc
import concourse.bass as bass
import concourse.mybir as mybir
import concourse.tile as tile
from concourse.bass import ts
from concourse.mybir import AluOpType, AxisListType
from concourse.bass_utils import run_bass_kernel_spmd

# problem shapes (hardcoded per contract)
DEPTH = 11
IN_W = 768
LEAF_W = 16
OUT_W = 768
N_NODES = 2047
N_LEAVES = 2048
BATCH = 8192
N_CORES = 8

BF = ml_dtypes.bfloat16

# routing kernel layout
B_CORE = BATCH // N_CORES            # 1024
DENSE_LEVELS = 8                     # levels 0..7 dense (255 nodes)
N_DENSE = 2 ** DENSE_LEVELS - 1      # 255
ND = 256                             # dense node columns (255 + pad)
KC = IN_W // 128                     # 6
EXTC = 772                           # gather row [w(768) | b | pad3]
DOT = IN_W + 1                       # 769 useful columns

# mlp kernel layout
LEAVES_PER_CORE = N_LEAVES // N_CORES           # 256
EXPERTS_PER_GROUP = 8
GROUPS = LEAVES_PER_CORE // EXPERTS_PER_GROUP   # 32
NH = OUT_W // 2                                 # 384 (psum bank half)
WSLAB_F = KC * 128 + OUT_W                      # 1536

F32 = mybir.dt.float32
BF16 = mybir.dt.bfloat16
I32 = mybir.dt.int32
I16 = mybir.dt.int16

LAST_PLAN = None  # (caps, offs, slots) of the most recent kernel() call


# ---------------------------------------------------------------- launch 1
def _build_routing_nc():
    nc = bacc.Bacc("TRN2", target_bir_lowering=False, debug=False,
                   num_devices=N_CORES)
    xT = nc.dram_tensor("xT", [IN_W, B_CORE], F32, kind="ExternalInput").ap()
    wd = nc.dram_tensor("wd", [IN_W, ND], F32, kind="ExternalInput").ap()
    ones = nc.dram_tensor("ones", [1, B_CORE], F32, kind="ExternalInput").ap()
    brow = nc.dram_tensor("brow", [1, ND], F32, kind="ExternalInput").ap()
    iot = nc.dram_tensor("iot", [128, ND], I16, kind="ExternalInput").ap()
    xe = nc.dram_tensor("xe", [B_CORE, DOT], F32, kind="ExternalInput").ap()
    nwe = nc.dram_tensor("nwe", [N_LEAVES, EXTC], F32, kind="ExternalInput").ap()
    leaf = nc.dram_tensor("leaf", [B_CORE], I32, kind="ExternalOutput").ap()

    with tile.TileContext(nc) as tc, contextlib.ExitStack() as ctx:
        pool = ctx.enter_context(tc.tile_pool(name="sbuf", bufs=1))
        wpool = ctx.enter_context(tc.tile_pool(name="work", bufs=2))
        psum = ctx.enter_context(tc.tile_pool(name="psum", bufs=1, space="PSUM"))

        # PE warmup: a stream of tiny matmuls keeps the cost model's p-state
        # ramp satisfied so the real matmuls run at full clock.
        wtiny = pool.tile([128, 8], F32)
        nc.vector.memset(wtiny[:], 0.0)
        ps = [psum.tile([128, ND], F32, space="PSUM", tag=f"ps{c}",
                        name=f"ps{c}") for c in range(8)]
        for i in range(60):
            nc.tensor.matmul(ps[0][:8, :8], lhsT=wtiny[:], rhs=wtiny[:],
                             start=(i == 0), stop=(i == 59))

        # loads: per-chunk tiles so matmul k can start as soon as its inputs
        # land; w chunks first (small), then x chunks (PE chases DMA)
        wd_r = wd.rearrange("(k p) n -> p k n", p=128)
        xT_r = xT.rearrange("(k p) s -> p k s", p=128)
        wd_sb, xT_sb = [], []
        ones_sb = pool.tile([1, B_CORE], F32)
        brow_sb = pool.tile([1, ND], F32)
        for k in range(KC):
            wd_sb.append(pool.tile([128, ND], F32, tag=f"wd{k}", name=f"wd{k}"))
            nc.sync.dma_start(out=wd_sb[k][:], in_=wd_r[:, k, :])
        nc.sync.dma_start(out=brow_sb[:], in_=brow[:])
        nc.sync.dma_start(out=ones_sb[:], in_=ones[:])
        for k in range(KC):
            xT_sb.append(pool.tile([128, B_CORE], F32, tag=f"xT{k}",
                                   name=f"xT{k}"))
            nc.sync.dma_start(out=xT_sb[k][:], in_=xT_r[:, k, :])
        iot_sb = pool.tile([128, ND], I16)
        nc.sync.dma_start(out=iot_sb[:], in_=iot[:])
        xe_sb = pool.tile([128, 8, DOT], F32)
        nc.sync.dma_start(out=xe_sb[:], in_=xe.rearrange("(c p) d -> p c d", p=128))

        # dense scores S[p, c, n] = x . w_n + b_n for nodes n in [0, 255)
        # (bias folded in as a 1-partition contraction row of ones).
        # K split in halves so wave 0 (c0..3) finishes early; the select walk
        # and gather chains start per 4-c wave while later work still runs.
        KH = KC // 2
        for k in range(KH):
            for c in range(8):
                nc.tensor.matmul(
                    ps[c][:], lhsT=xT_sb[k][:, ts(c, 128)], rhs=wd_sb[k][:],
                    start=(k == 0), stop=False,
                )
        s_w = [pool.tile([128, 4, ND], BF16, tag=f"s{w}", name=f"s{w}")
               for w in range(2)]
        for w in range(2):
            for i in range(4):
                c = 4 * w + i
                for k in range(KH, KC):
                    nc.tensor.matmul(
                        ps[c][:], lhsT=xT_sb[k][:, ts(c, 128)], rhs=wd_sb[k][:],
                        start=False, stop=False,
                    )
                nc.tensor.matmul(
                    ps[c][:], lhsT=ones_sb[:, ts(c, 128)], rhs=brow_sb[:],
                    start=False, stop=True,
                )
                # PSUM->SBUF score copy on the otherwise-idle Scalar engine
                nc.scalar.copy(out=s_w[w][:, i, :], in_=ps[c][:])

        # select walk, levels 0..7, per wave. cur tracks node_id + 1 so the
        # update is cur = 2*cur + signbit; iot holds node_id + 1. i16, 2x DVE.
        sgn_w, cur_w = [], []
        junk = {c: pool.tile([128, DOT], F32, tag=f"junk{c}", name=f"junk{c}")
                for c in range(8)}
        pairs = [(q, slice(2 * (q % 2), 2 * (q % 2) + 2)) for q in range(4)]
        cur_q, sc_q, ch_q, leaf_q, gath_q = {}, {}, {}, {}, {}

        def emit_select_wave(w):
            sgn = pool.tile([128, 4, ND], I16, tag=f"sgn{w}", name=f"sgnw{w}")
            nc.vector.tensor_scalar(out=sgn[:], in0=s_w[w][:], scalar1=0.0,
                                    scalar2=None, op0=AluOpType.is_ge)
            cur = pool.tile([128, 4], I16, tag=f"curw{w}", name=f"curw{w}")
            sel = pool.tile([128, 4], I16, tag=f"selw{w}", name=f"selw{w}")
            mask = pool.tile([128, 4, 128], I16, tag=f"mk{w}", name=f"mkw{w}")
            prod = pool.tile([128, 4, 128], I16, tag=f"pd{w}", name=f"pdw{w}")
            nc.vector.tensor_scalar(out=cur[:], in0=sgn[:, :, 0], scalar1=2,
                                    scalar2=None, op0=AluOpType.add)
            for lvl in range(1, DENSE_LEVELS):
                n = 2 ** lvl
                off = n - 1
                nc.vector.tensor_tensor(
                    out=mask[:, :, :n],
                    in0=iot_sb[:, None, off:off + n].to_broadcast([128, 4, n]),
                    in1=cur[:, :, None].to_broadcast([128, 4, n]),
                    op=AluOpType.is_equal,
                )
                nc.vector.tensor_tensor(
                    out=prod[:, :, :n], in0=mask[:, :, :n],
                    in1=sgn[:, :, off:off + n], op=AluOpType.mult,
                )
                with nc.allow_low_precision(reason="one-hot i16 reduce, exact"):
                    nc.vector.tensor_reduce(out=sel[:], in_=prod[:, :, :n],
                                            axis=AxisListType.X, op=AluOpType.add)
                nc.vector.scalar_tensor_tensor(out=cur[:], in0=cur[:], scalar=2,
                                               in1=sel[:], op0=AluOpType.mult,
                                               op1=AluOpType.add)
            sgn_w.append(sgn)
            cur_w.append(cur)

        def emit_gather(q, lvl, cur_src):
            idx32 = wpool.tile([128, 2], I32, tag=f"idx{q}", name=f"idx{q}l{lvl}")
            # cur tracks node_id + 1; indirect offsets want node_id
            nc.vector.tensor_scalar(out=idx32[:], in0=cur_src,
                                    scalar1=1, scalar2=None,
                                    op0=AluOpType.subtract)
            gs = []
            for ci in range(2):
                c = 2 * q + ci
                g = wpool.tile([128, EXTC], F32, tag=f"g{c}", name=f"g{c}l{lvl}")
                nc.gpsimd.indirect_dma_start(
                    out=g[:], out_offset=None, in_=nwe[:],
                    in_offset=bass.IndirectOffsetOnAxis(
                        ap=idx32[:, ci:ci + 1], axis=0),
                )
                gs.append(g)
            gath_q[q] = gs

        def emit_dots_update(q, lvl, cur_src):
            for ci in range(2):
                c = 2 * q + ci
                nc.vector.scalar_tensor_tensor(
                    out=junk[c][:], in0=xe_sb[:, c, :DOT], scalar=1.0,
                    in1=gath_q[q][ci][:, :DOT], op0=AluOpType.mult,
                    op1=AluOpType.mult, accum_out=sc_q[q][:, ci:ci + 1],
                )
            nc.vector.tensor_scalar(out=ch_q[q][:], in0=sc_q[q][:],
                                    scalar1=0.0, scalar2=None,
                                    op0=AluOpType.is_ge)
            nc.vector.scalar_tensor_tensor(
                out=cur_q[q][:], in0=cur_src, scalar=2, in1=ch_q[q][:],
                op0=AluOpType.mult, op1=AluOpType.add)

        for q in range(4):
            cur_q[q] = pool.tile([128, 2], I16, tag=f"cur{q}", name=f"cur{q}")
            sc_q[q] = pool.tile([128, 2], F32, tag=f"sc{q}", name=f"sc{q}")
            ch_q[q] = pool.tile([128, 2], I16, tag=f"ch{q}", name=f"ch{q}")

        # wave 0 select, then its level-8 gathers; wave 1 select overlaps the
        # gather DMA latency; chains then proceed level-major.
        emit_select_wave(0)
        # pairs 0,1 <- wave 0 (c0..3); pairs 2,3 <- wave 1 (c4..7)
        emit_gather(0, DENSE_LEVELS, cur_w[0][:, 0:2])
        emit_gather(1, DENSE_LEVELS, cur_w[0][:, 2:4])
        emit_select_wave(1)
        emit_gather(2, DENSE_LEVELS, cur_w[1][:, 0:2])
        emit_gather(3, DENSE_LEVELS, cur_w[1][:, 2:4])

        def cur_src_of(q, lvl):
            if lvl == DENSE_LEVELS:
                return cur_w[q // 2][:, 2 * (q % 2):2 * (q % 2) + 2]
            return cur_q[q][:]

        for lvl in range(DENSE_LEVELS, DEPTH):
            for q in range(4):
                emit_dots_update(q, lvl, cur_src_of(q, lvl))
                if lvl + 1 < DEPTH:
                    emit_gather(q, lvl + 1, cur_q[q][:])

        leaf_i = pool.tile([128, 8], I32)
        for q in range(4):
            csl = slice(2 * q, 2 * q + 2)
            leaf_q[q] = pool.tile([128, 2], I16, tag=f"lf{q}", name=f"lf{q}")
            nc.vector.tensor_scalar(out=leaf_q[q][:], in0=cur_q[q][:],
                                    scalar1=N_NODES + 1, scalar2=None,
                                    op0=AluOpType.subtract)
            nc.vector.tensor_copy(out=leaf_i[:, csl], in_=leaf_q[q][:])
        nc.sync.dma_start(out=leaf.rearrange("(c p) -> p c", p=128), in_=leaf_i[:])

    nc.compile()
    return nc


def _host_prep_routing(x, node_weights, node_biases):
    wd = np.zeros((IN_W, ND), np.float32)
    wd[:, :N_DENSE] = node_weights[:N_DENSE].T
    brow = np.zeros((1, ND), np.float32)
    brow[0, :N_DENSE] = node_biases[:N_DENSE]
    ones = np.ones((1, B_CORE), np.float32)
    # iot holds node_id + 1 (the select walk tracks cur = node_id + 1)
    iot = np.tile(np.arange(1, ND + 1, dtype=np.int16)[None, :], (128, 1))
    nwe = np.zeros((N_LEAVES, EXTC), np.float32)
    nwe[:N_NODES, :IN_W] = node_weights
    nwe[:N_NODES, IN_W] = node_biases

    in_maps = []
    for c in range(N_CORES):
        xs = x[c * B_CORE:(c + 1) * B_CORE]
        xT = np.ascontiguousarray(xs.T)
        xev = np.empty((B_CORE, DOT), np.float32)
        xev[:, :IN_W] = xs
        xev[:, IN_W] = 1.0
        in_maps.append({"xT": xT, "wd": wd, "ones": ones, "brow": brow,
                        "iot": iot, "xe": xev, "nwe": nwe})
    return in_maps


# ---------------------------------------------------------------- launch 2
def _build_mlp_nc(caps):
    caps = list(caps)
    offs = np.concatenate([[0], np.cumsum(caps)]).astype(int)
    slots = int(offs[-1])
    slots_pad = -(-slots // 8) * 8

    nc = bacc.Bacc("TRN2", target_bir_lowering=False, debug=False,
                   num_devices=N_CORES)
    xgT = nc.dram_tensor("xgT", [IN_W, slots_pad], BF16, kind="ExternalInput").ap()
    wslab = nc.dram_tensor("wslab", [GROUPS, 128, WSLAB_F], BF16,
                           kind="ExternalInput").ap()
    b1bc = nc.dram_tensor("b1bc", [128, GROUPS], F32, kind="ExternalInput").ap()
    maskt = nc.dram_tensor("maskt", [128, slots_pad], BF16, kind="ExternalInput").ap()
    out = nc.dram_tensor("o", [slots_pad, OUT_W], BF16, kind="ExternalOutput").ap()

    # static out-packing plan: greedy fill of <=128-row packs
    packs = []  # list of (groups, rows)
    cg, rows = [], 0
    for g in range(GROUPS):
        if caps[g] == 0:
            continue
        if rows + caps[g] > 128:
            packs.append((cg, rows))
            cg, rows = [], 0
        cg.append(g)
        rows += caps[g]
    if cg:
        packs.append((cg, rows))

    with tile.TileContext(nc) as tc, contextlib.ExitStack() as ctx:
        pool = ctx.enter_context(tc.tile_pool(name="sbuf", bufs=1))
        wpool = ctx.enter_context(tc.tile_pool(name="w", bufs=4))
        hpool = ctx.enter_context(tc.tile_pool(name="h", bufs=3))
        opool = ctx.enter_context(tc.tile_pool(name="o", bufs=3))
        ps1 = ctx.enter_context(tc.tile_pool(name="ps1", bufs=3, space="PSUM"))
        ps2 = ctx.enter_context(tc.tile_pool(name="ps2", bufs=2, space="PSUM"))

        # PE warmup stream (see routing builder)
        wtiny = pool.tile([128, 8], BF16)
        nc.vector.memset(wtiny[:], 0.0)
        wps = ps1.tile([8, 8], F32, space="PSUM", tag="p1", name="wps")
        for i in range(60):
            nc.tensor.matmul(wps[:], lhsT=wtiny[:], rhs=wtiny[:],
                             start=(i == 0), stop=(i == 59))

        WCHUNK = 4  # groups per weight DMA
        w_r = wslab.rearrange("(u g) p f -> u p g f", g=WCHUNK)

        xt_r = xgT.rearrange("(k p) s -> p k s", p=128)
        # interleave: first weight chunk, then xt, then the rest just-in-time
        w_tiles = {}
        w_tiles[0] = wpool.tile([128, WCHUNK, WSLAB_F], BF16, tag="w", name="w0")
        nc.sync.dma_start(out=w_tiles[0][:], in_=w_r[0])
        xt_sb = []
        for k in range(KC):
            xt_sb.append(pool.tile([128, slots_pad], BF16, tag=f"xt{k}",
                                   name=f"xt{k}"))
            nc.sync.dma_start(out=xt_sb[k][:], in_=xt_r[:, k, :])
        b1_sb = pool.tile([128, GROUPS], F32)
        nc.sync.dma_start(out=b1_sb[:], in_=b1bc[:])
        mask_sb = pool.tile([128, slots_pad], BF16)
        nc.sync.dma_start(out=mask_sb[:], in_=maskt[:])

        ncopy = 0
        for pi, (groups, rows) in enumerate(packs):
            o_sb = opool.tile([128, OUT_W], BF16, tag="opack", name=f"opack{pi}")
            r0 = 0
            for g in groups:
                u, gi = g // WCHUNK, g % WCHUNK
                if u not in w_tiles:
                    w_tiles[u] = wpool.tile([128, WCHUNK, WSLAB_F], BF16,
                                            tag="w", name=f"w{u}")
                    nc.sync.dma_start(out=w_tiles[u][:], in_=w_r[u])
                w_sb = w_tiles[u]
                w1_sb = w_sb[:, gi, :KC * 128].rearrange("p (k n) -> p k n", k=KC)
                w2_sb = w_sb[:, gi, KC * 128:]

                cap = caps[g]
                sl = slice(int(offs[g]), int(offs[g]) + cap)
                p1 = ps1.tile([128, cap], F32, space="PSUM", tag="p1", name=f"p1g{g}")
                for k in range(KC):
                    nc.tensor.matmul(
                        p1[:], lhsT=w1_sb[:, k, :], rhs=xt_sb[k][:, sl],
                        start=(k == 0), stop=(k == KC - 1),
                    )
                hr = hpool.tile([128, cap], BF16, tag="hr", name=f"hrg{g}")
                nc.scalar.activation(
                    out=hr[:], in_=p1[:], func=mybir.ActivationFunctionType.Relu,
                    bias=b1_sb[:, g:g + 1], scale=1.0,
                )
                hf = hpool.tile([128, cap], BF16, tag="hf", name=f"hfg{g}")
                nc.vector.tensor_tensor(out=hf[:], in0=hr[:],
                                        in1=mask_sb[:, sl], op=AluOpType.mult)

                p2a = ps2.tile([cap, NH], F32, space="PSUM", tag="p2a", name=f"p2ag{g}")
                p2b = ps2.tile([cap, NH], F32, space="PSUM", tag="p2b", name=f"p2bg{g}")
                nc.tensor.matmul(p2a[:], lhsT=hf[:], rhs=w2_sb[:, :NH],
                                 start=True, stop=True)
                nc.tensor.matmul(p2b[:], lhsT=hf[:], rhs=w2_sb[:, NH:],
                                 start=True, stop=True)
                for half, p2 in ((0, p2a), (1, p2b)):
                    eng = nc.vector if ncopy % 2 == 0 else nc.scalar
                    dst = o_sb[r0:r0 + cap, half * NH:(half + 1) * NH]
                    if eng is nc.vector:
                        nc.vector.tensor_copy(out=dst, in_=p2[:])
                    else:
                        nc.scalar.copy(out=dst, in_=p2[:])
                    ncopy += 1
                r0 += cap
            obase = int(offs[groups[0]])
            nc.sync.dma_start(out=out[obase:obase + rows, :], in_=o_sb[:rows, :])

    nc.compile()
    return nc


def _plan_slots(leaves):
    """Shared capacity plan: per core sort groups by occupancy (desc); rank i
    capacity = max over cores of i-th largest count."""
    counts = np.zeros((N_CORES, GROUPS), np.int64)
    for c in range(N_CORES):
        lo = LEAVES_PER_CORE * c
        sel = (leaves >= lo) & (leaves < lo + LEAVES_PER_CORE)
        counts[c] = np.bincount((leaves[sel] - lo) // EXPERTS_PER_GROUP,
                                minlength=GROUPS)
    order = np.argsort(-counts, axis=1, kind="stable")  # [core, rank] -> group
    sorted_counts = -np.sort(-counts, axis=1)
    caps = sorted_counts.max(axis=0)  # [rank]
    assert caps[0] <= 128, f"group overflow: {caps[0]}"
    return counts, order, caps


def _host_prep_mlp(leaves, x, w1s, b1s, w2s, order, caps):
    offs = np.concatenate([[0], np.cumsum(caps)]).astype(int)
    slots = int(offs[-1])
    slots_pad = -(-slots // 8) * 8

    in_maps, slot_maps = [], []
    for c in range(N_CORES):
        lo = LEAVES_PER_CORE * c
        sel = np.nonzero((leaves >= lo) & (leaves < lo + LEAVES_PER_CORE))[0]
        l_loc = leaves[sel] - lo
        g_all = l_loc // EXPERTS_PER_GROUP
        e_all = l_loc % EXPERTS_PER_GROUP
        rank_of = np.empty(GROUPS, np.int64)
        rank_of[order[c]] = np.arange(GROUPS)
        r_all = rank_of[g_all]
        slot = np.empty(len(sel), np.int64)
        fill = np.zeros(GROUPS, np.int64)
        for i, r in enumerate(r_all):
            slot[i] = offs[r] + fill[r]
            fill[r] += 1

        slot_sample = np.full(slots_pad, -1, np.int64)
        slot_sample[slot] = sel
        mask = np.zeros((128, slots_pad), BF)
        lane_rows = (16 * e_all[None, :] + np.arange(16)[:, None])
        mask[lane_rows, slot[None, :]] = 1.0

        xg = np.zeros((slots_pad, IN_W), np.float32)
        xg[slot] = x[sel]
        xgT = np.ascontiguousarray(xg.T).astype(BF)

        ginv = order[c]  # rank -> group
        gsel = ginv * EXPERTS_PER_GROUP + lo  # leaf base per rank
        w1f = np.stack([
            w1s[gsel[r]:gsel[r] + 8]                       # [8, 768, 16]
            .transpose(1, 0, 2).reshape(IN_W, 128)         # [768, 128]
            .reshape(KC, 128, 128).transpose(1, 0, 2)      # [128, KC, 128]
            .reshape(128, KC * 128)
            for r in range(GROUPS)
        ])                                                  # [G, 128, 768]
        w2f = np.stack([
            w2s[gsel[r]:gsel[r] + 8].reshape(128, OUT_W) for r in range(GROUPS)
        ])
        wslab = np.concatenate([w1f, w2f], axis=2).astype(BF)
        b1v = np.stack([b1s[gsel[r]:gsel[r] + 8].reshape(128) for r in range(GROUPS)])
        b1bc = np.ascontiguousarray(b1v.T).astype(np.float32)

        in_maps.append({"xgT": xgT, "wslab": wslab, "b1bc": b1bc, "maskt": mask})
        slot_maps.append(slot_sample)
    return in_maps, slot_maps


# ---------------------------------------------------------------- entry
def kernel(x, node_weights, node_biases, w1s, b1s, w2s):
    x = np.ascontiguousarray(np.asarray(x, np.float32))
    node_weights = np.ascontiguousarray(np.asarray(node_weights, np.float32))
    node_biases = np.ascontiguousarray(np.asarray(node_biases, np.float32))
    w1s = np.asarray(w1s, np.float32)
    b1s = np.asarray(b1s, np.float32)
    w2s = np.asarray(w2s, np.float32)

    # launch 1: routing
    nc1 = _build_routing_nc()
    in1 = _host_prep_routing(x, node_weights, node_biases)
    res1 = run_bass_kernel_spmd(nc1, in1, core_ids=list(range(N_CORES)))
    leaves = np.concatenate([res1.results[c]["leaf"] for c in range(N_CORES)])
    leaves = leaves.astype(np.int64)

    # launch 2: expert MLP with shared sorted-capacity plan
    counts, order, caps = _plan_slots(leaves)
    global LAST_PLAN
    LAST_PLAN = caps
    nc2 = _build_mlp_nc(caps)
    in2, slot_maps = _host_prep_mlp(leaves, x, w1s, b1s, w2s, order, caps)
    res2 = run_bass_kernel_spmd(nc2, in2, core_ids=list(range(N_CORES)))

    out = np.zeros((BATCH, OUT_W), np.float32)
    for c in range(N_CORES):
        o_slots = np.asarray(res2.results[c]["o"], dtype=np.float32)
        sm = slot_maps[c]
        valid = sm >= 0
        out[sm[valid]] = o_slots[valid]
    return out
